# revision 7
# baseline (speedup 1.0000x reference)
"""Trainium2 Bass kernel for a Matching Network attention head.

Reference computation (see problem statement):
    q_proj = query @ W1[:D]                       # [Q, D]
    s_proj = support @ W1[D:]                     # [S, D]
    hidden = relu(q_proj[:,None,:] + s_proj[None,:,:] + b1)   # [Q, S, D]
    scores = einsum('qsd,d->qs', hidden, W2) + b2
    weights = softmax(scores, axis=1)
    logits  = weights @ onehot(support_labels)    # [Q, n_way]

Sharding strategy (8 cores): shard the SUPPORT set (40 of 320 rows per
core) and replicate the queries.  Each core produces the *unnormalized*
partial numerators and denominator of the softmax-weighted average:

    part[w, q]  = sum_{s in shard} exp(score[s,q]) * onehot[s,w]   (w < 20)
    part[20, q] = sum_{s in shard} exp(score[s,q])

The host sums the partials over cores and divides - softmax over the
full support set falls out exactly (b2 is a constant shift over s and
cancels in the softmax, so it is dropped).  exp() is computed without a
max-subtraction: scores are ~N(0, 0.7) for this problem so fp32 exp is
safe and exact.

Per-core device program:
  - qpT[dout, q]  = W1a^T @ queryT      (PE, bf16, fp32 psum)
  - spbT[dout, s] = W1b^T @ supportT + b1   (PE + DVE add)
  - For each s: H = relu(qpT + spbT[:, s]) as a single fused
    tensor_scalar(add, max) on DVE (bf16 in/out -> 4x mode) or an
    activation(Relu, bias) on ACT; the 80 ops are split ~62/18 so both
    engines finish together.
  - scores[s, q] = sum_d W2[d] * H[d, q] via one-hot-column matmuls:
    lhsT is [128, 32] with W2's d-block in column (s//4), output goes to
    psum partitions [32*(s%4) .. +32).  Writing to base partition 32*j
    makes bass emit tile_position=(0, 32*j) so 4 consecutive matmuls run
    concurrently in distinct 32-column groups of the PE array.
  - E = exp(scores) on ACT (psum -> sbuf, bf16)
  - part = [onehot | ones]^T @ E on PE, copied out as [21, Q] fp32.
"""

import numpy as np
import ml_dtypes

bf16 = ml_dtypes.bfloat16

N_CORES = 8
Q, D, S, NWAY = 2048, 256, 320, 20
SP = S // N_CORES          # 40 support rows per core
NQC = 4                    # q chunks of 512 (one psum bank each)
QC = Q // NQC
NR = SP // 4               # 10 rounds of 4 concurrent s-values

_compiled = None           # (nc, run) cache


def _build_nc():
    import concourse.bass as bass
    import concourse.tile as tile
    from concourse import mybir

    f32 = mybir.dt.float32
    b16 = mybir.dt.bfloat16
    RELU = mybir.ActivationFunctionType.Relu
    EXP = mybir.ActivationFunctionType.Exp
    ADD = mybir.AluOpType.add
    MAX = mybir.AluOpType.max

    nc = bass.Bass()
    qT_d = nc.declare_dram_parameter("qT", [D, Q], b16, isOutput=False)
    w1a_d = nc.declare_dram_parameter("w1a", [D, D], b16, isOutput=False)
    w1b_d = nc.declare_dram_parameter("w1b", [D, D], b16, isOutput=False)
    sT_d = nc.declare_dram_parameter("sT", [D, SP], b16, isOutput=False)
    b1_d = nc.declare_dram_parameter("b1c", [128, 2], f32, isOutput=False)
    w2c_d = nc.declare_dram_parameter("w2c", [128, 2 * NR * 32], b16, isOutput=False)
    ohm_d = nc.declare_dram_parameter("ohm", [128, NWAY + 1], b16, isOutput=False)
    out_d = nc.declare_dram_parameter("part", [NWAY + 1, Q], f32, isOutput=True)

    with tile.TileContext(nc) as tc:
        with (
            tc.tile_pool(name="const", bufs=1) as cpool,
            tc.tile_pool(name="stage", bufs=1) as spool,
            tc.tile_pool(name="hpool", bufs=16) as hpool,
            tc.tile_pool(name="psum", bufs=4, space="PSUM") as ppool,
        ):
            # ---- load constants / inputs ------------------------------
            w1a_t = [cpool.tile([128, D], b16, name=f"w1a{i}") for i in range(2)]
            w1b_t = [cpool.tile([128, D], b16, name=f"w1b{i}") for i in range(2)]
            sT_t = [cpool.tile([128, SP], b16, name=f"sT{i}") for i in range(2)]
            qT_t = [spool.tile([128, Q], b16, name=f"qTt{i}") for i in range(2)]
            b1_t = cpool.tile([128, 2], f32, name="b1t")
            w2c_t = cpool.tile([128, 2 * NR * 32], b16, name="w2ct")
            ohm_t = cpool.tile([128, NWAY + 1], b16, name="ohmt")
            for i in range(2):
                nc.sync.dma_start(out=qT_t[i][:], in_=qT_d[128 * i : 128 * (i + 1)])
                nc.sync.dma_start(out=w1a_t[i][:], in_=w1a_d[128 * i : 128 * (i + 1)])
                nc.sync.dma_start(out=w1b_t[i][:], in_=w1b_d[128 * i : 128 * (i + 1)])
                nc.sync.dma_start(out=sT_t[i][:], in_=sT_d[128 * i : 128 * (i + 1)])
            nc.sync.dma_start(out=b1_t[:], in_=b1_d[:])
            nc.sync.dma_start(out=w2c_t[:], in_=w2c_d[:])
            nc.sync.dma_start(out=ohm_t[:], in_=ohm_d[:])

            # ---- spbT = W1b^T @ supportT + b1   [2][128, SP] f32 ------
            spb_t = [cpool.tile([128, SP], f32, name=f"spb{i}") for i in range(2)]
            for db in range(2):
                sps = ppool.tile([128, QC], f32, tag="ps", name=f"sps{db}")
                nc.tensor.matmul(
                    sps[:, :SP],
                    w1b_t[0][:, 128 * db : 128 * (db + 1)],
                    sT_t[0][:],
                    start=True, stop=False,
                )
                nc.tensor.matmul(
                    sps[:, :SP],
                    w1b_t[1][:, 128 * db : 128 * (db + 1)],
                    sT_t[1][:],
                    start=False, stop=True,
                )
                nc.vector.tensor_scalar(
                    out=spb_t[db][:], in0=sps[:, :SP],
                    scalar1=b1_t[:, db : db + 1], scalar2=None, op0=ADD,
                )

            # ---- qpT = W1a^T @ queryT   [2][128, Q] bf16 --------------
            qpT_t = [spool.tile([128, Q], b16, name=f"qpT{i}") for i in range(2)]
            ncopy = 0
            for db in range(2):
                for qc in range(NQC):
                    qps = ppool.tile([128, QC], f32, tag="ps", name=f"qps{db}{qc}")
                    nc.tensor.matmul(
                        qps[:],
                        w1a_t[0][:, 128 * db : 128 * (db + 1)],
                        qT_t[0][:, QC * qc : QC * (qc + 1)],
                        start=True, stop=False,
                    )
                    nc.tensor.matmul(
                        qps[:],
                        w1a_t[1][:, 128 * db : 128 * (db + 1)],
                        qT_t[1][:, QC * qc : QC * (qc + 1)],
                        start=False, stop=True,
                    )
                    dst = qpT_t[db][:, QC * qc : QC * (qc + 1)]
                    if ncopy % 2 == 0:
                        nc.vector.tensor_copy(out=dst, in_=qps[:])
                    else:
                        nc.scalar.copy(out=dst, in_=qps[:])
                    ncopy += 1

            # ---- main loop: H = relu(qpT + spb[:, s]);  scores += ----
            scores_ps = [
                ppool.tile([128, QC], f32, tag="ps", name=f"sc{qc}") for qc in range(NQC)
            ]
            ts_idx = 0
            for r in range(NR):
                h_tiles = {}
                for j in range(4):
                    sl = 4 * r + j
                    for db in range(2):
                        h = hpool.tile([128, Q], b16, tag="H", name=f"h{sl}_{db}")
                        if ts_idx % 9 >= 7:   # ~2/9 of ops on ACT
                            nc.scalar.activation(
                                h[:], qpT_t[db][:], RELU,
                                bias=spb_t[db][:, sl : sl + 1],
                            )
                        else:
                            nc.vector.tensor_scalar(
                                out=h[:], in0=qpT_t[db][:],
                                scalar1=spb_t[db][:, sl : sl + 1],
                                scalar2=0.0, op0=ADD, op1=MAX,
                            )
                        ts_idx += 1
                        h_tiles[(j, db)] = h
                for db in range(2):
                    w2blk = w2c_t[:, 32 * (db * NR + r) : 32 * (db * NR + r) + 32]
                    for qc in range(NQC):
                        for j in range(4):
                            nc.tensor.matmul(
                                scores_ps[qc][32 * j : 32 * j + 32, :],
                                w2blk,
                                h_tiles[(j, db)][:, QC * qc : QC * (qc + 1)],
                                start=(r == 0 and db == 0),
                                stop=(r == NR - 1 and db == 1),
                                tile_position=(0, 32 * j),
                                skip_group_check=True,
                            )

            # ---- E = exp(scores)  [128, Q] bf16 -----------------------
            e_t = spool.tile([128, Q], b16, name="et")
            for qc in range(NQC):
                nc.scalar.activation(
                    e_t[:, QC * qc : QC * (qc + 1)], scores_ps[qc][:], EXP,
                )

            # ---- part = [onehot | ones]^T @ E  -> [21, Q] f32 ---------
            out_sb = spool.tile([NWAY + 1, Q], f32, name="outsb")
            for qc in range(NQC):
                fps = ppool.tile([NWAY + 1, QC], f32, tag="ps", name=f"fps{qc}")
                nc.tensor.matmul(
                    fps[:], ohm_t[:], e_t[:, QC * qc : QC * (qc + 1)],
                    start=True, stop=True,
                )
                dst = out_sb[:, QC * qc : QC * (qc + 1)]
                if qc % 2 == 0:
                    nc.vector.tensor_copy(out=dst, in_=fps[:])
                else:
                    nc.scalar.copy(out=dst, in_=fps[:])
            nc.sync.dma_start(out=out_d[:], in_=out_sb[:])

    nc.finalize()
    return nc


def _host_prep(inputs):
    """Host-side layout prep: transposes, dtype casts, one-hot tables.

    Returns the list of 8 per-core input dicts for the bass kernel.
    """
    q = np.ascontiguousarray(np.asarray(inputs["query_embeddings"], dtype=np.float32))
    s = np.ascontiguousarray(np.asarray(inputs["support_embeddings"], dtype=np.float32))
    lab = np.asarray(inputs["support_labels"]).astype(np.int64)
    W1 = np.asarray(inputs["W1"], dtype=np.float32)
    b1 = np.asarray(inputs["b1"], dtype=np.float32)
    W2 = np.asarray(inputs["W2"], dtype=np.float32)

    qT = np.ascontiguousarray(q.T).astype(bf16)            # [D, Q]
    sT_full = np.ascontiguousarray(s.T).astype(bf16)       # [D, S]
    w1a = np.ascontiguousarray(W1[:D]).astype(bf16)        # [D, D] (din, dout)
    w1b = np.ascontiguousarray(W1[D:]).astype(bf16)
    b1c = np.ascontiguousarray(b1.reshape(2, 128).T).astype(np.float32)  # [128, 2]

    w2c = np.zeros((128, 2 * NR * 32), dtype=np.float32)
    for db in range(2):
        blk = W2[128 * db : 128 * (db + 1)]
        for r in range(NR):
            w2c[:, 32 * (db * NR + r) + r] = blk
    w2c = w2c.astype(bf16)

    in_maps = []
    for c in range(N_CORES):
        lo = c * SP
        ohm = np.zeros((128, NWAY + 1), dtype=np.float32)
        for sl in range(SP):
            row = 32 * (sl % 4) + sl // 4
            ohm[row, lab[lo + sl]] = 1.0
            ohm[row, NWAY] = 1.0
        in_maps.append(
            {
                "qT": qT,
                "w1a": w1a,
                "w1b": w1b,
                "sT": np.ascontiguousarray(sT_full[:, lo : lo + SP]),
                "b1c": b1c,
                "w2c": w2c,
                "ohm": ohm.astype(bf16),
            }
        )
    return in_maps


def _combine(parts):
    """Sum per-core partials and normalize -> [Q, NWAY] f32."""
    total = np.zeros((NWAY + 1, Q), dtype=np.float32)
    for p in parts:
        total += np.asarray(p, dtype=np.float32)
    return np.ascontiguousarray((total[:NWAY] / total[NWAY : NWAY + 1]).T)


def get_nc():
    global _compiled
    if _compiled is None:
        _compiled = _build_nc()
    return _compiled


def kernel(**inputs) -> np.ndarray:
    from concourse.bass_utils import run_bass_kernel_spmd

    nc = get_nc()
    in_maps = _host_prep(inputs)
    res = run_bass_kernel_spmd(nc, in_maps, list(range(N_CORES)))
    return _combine([res.results[c]["part"] for c in range(N_CORES)])


# revision 16
# speedup vs baseline: 1.1843x; 1.1843x over previous
"""Trainium2 Bass kernel for a Matching Network attention head.

Reference computation (see problem statement):
    q_proj = query @ W1[:D]                       # [Q, D]
    s_proj = support @ W1[D:]                     # [S, D]
    hidden = relu(q_proj[:,None,:] + s_proj[None,:,:] + b1)   # [Q, S, D]
    scores = einsum('qsd,d->qs', hidden, W2) + b2
    weights = softmax(scores, axis=1)
    logits  = weights @ onehot(support_labels)    # [Q, n_way]

Sharding strategy (8 cores): shard the SUPPORT set (40 of 320 rows per
core) and replicate the queries.  Each core produces the *unnormalized*
partial numerators and denominator of the softmax-weighted average:

    part[w, q]  = sum_{s in shard} exp(score[s,q]) * onehot[s,w]   (w < 20)
    part[20, q] = sum_{s in shard} exp(score[s,q])

The host sums the partials over cores and divides - softmax over the
full support set falls out exactly (b2 is a constant shift over s and
cancels in the softmax, so it is dropped).  exp() is computed without a
max-subtraction: scores are ~N(0, 0.7) for this problem so fp32 exp is
safe and exact.

Per-core device program:
  - qpT[dout, q]  = W1a^T @ queryT      (PE, bf16, fp32 psum)
  - spbT[dout, s] = W1b^T @ supportT + b1   (PE + DVE add)
  - For each s: H = relu(qpT + spbT[:, s]) as a single fused
    tensor_scalar(add, max) on DVE (bf16 in/out -> 4x mode) or an
    activation(Relu, bias) on ACT; the 80 ops are split ~62/18 so both
    engines finish together.
  - scores[s, q] = sum_d W2[d] * H[d, q] via one-hot-column matmuls:
    lhsT is [128, 32] with W2's d-block in column (s//4), output goes to
    psum partitions [32*(s%4) .. +32).  Writing to base partition 32*j
    makes bass emit tile_position=(0, 32*j) so 4 consecutive matmuls run
    concurrently in distinct 32-column groups of the PE array.
  - E = exp(scores) on ACT (psum -> sbuf, bf16)
  - part = [onehot | ones]^T @ E on PE, copied out as [21, Q] fp32.
"""

import numpy as np
import ml_dtypes

bf16 = ml_dtypes.bfloat16

N_CORES = 8
Q, D, S, NWAY = 2048, 256, 320, 20
SP = S // N_CORES          # 40 support rows per core
NQC = 4                    # q chunks of 512 (one psum bank each)
QC = Q // NQC
NR = SP // 4               # 10 rounds of 4 concurrent s-values

_compiled = None           # (nc, run) cache


def _build_nc():
    import concourse.tile as tile
    from concourse import mybir
    from concourse.bacc import Bacc

    f32 = mybir.dt.float32
    b16 = mybir.dt.bfloat16
    RELU = mybir.ActivationFunctionType.Relu
    EXP = mybir.ActivationFunctionType.Exp
    ADD = mybir.AluOpType.add
    MAX = mybir.AluOpType.max

    nc = Bacc()
    qT_d = nc.declare_dram_parameter("qT", [D, Q], b16, isOutput=False)
    w1a_d = nc.declare_dram_parameter("w1a", [D, D], b16, isOutput=False)
    w1b_d = nc.declare_dram_parameter("w1b", [D, D], b16, isOutput=False)
    sT_d = nc.declare_dram_parameter("sT", [D, SP], b16, isOutput=False)
    b1_d = nc.declare_dram_parameter("b1r", [1, D], b16, isOutput=False)
    w2c_d = nc.declare_dram_parameter("w2c", [128, 2 * NR * 32], b16, isOutput=False)
    ohm_d = nc.declare_dram_parameter("ohm", [128, NWAY + 1], b16, isOutput=False)
    out_d = nc.declare_dram_parameter("part", [NWAY + 1, Q], f32, isOutput=True)

    with tile.TileContext(nc) as tc:
        with (
            tc.tile_pool(name="const", bufs=1) as cpool,
            tc.tile_pool(name="stage", bufs=1) as spool,
            tc.tile_pool(name="hpool", bufs=16) as hpool,
            tc.tile_pool(name="psum", bufs=4, space="PSUM") as ppool,
        ):
            # ---- load constants / inputs ------------------------------
            w1a_t = [cpool.tile([128, D], b16, name=f"w1a{i}") for i in range(2)]
            w1b_t = [cpool.tile([128, D], b16, name=f"w1b{i}") for i in range(2)]
            sT_t = [cpool.tile([128, SP], b16, name=f"sT{i}") for i in range(2)]
            qT_t = [spool.tile([128, Q], b16, name=f"qTt{i}") for i in range(2)]
            b1_t = cpool.tile([1, D], b16, name="b1t")
            ones_t = cpool.tile([1, SP], b16, name="onest")
            nc.vector.memset(ones_t[:], 1.0)
            w2c_t = cpool.tile([128, 2 * NR * 32], b16, name="w2ct")
            ohm_t = cpool.tile([128, NWAY + 1], b16, name="ohmt")
            for i in range(2):
                nc.sync.dma_start(out=qT_t[i][:], in_=qT_d[128 * i : 128 * (i + 1)])
                nc.sync.dma_start(out=w1a_t[i][:], in_=w1a_d[128 * i : 128 * (i + 1)])
                nc.sync.dma_start(out=w1b_t[i][:], in_=w1b_d[128 * i : 128 * (i + 1)])
                nc.sync.dma_start(out=sT_t[i][:], in_=sT_d[128 * i : 128 * (i + 1)])
            nc.sync.dma_start(out=b1_t[:], in_=b1_d[:])
            nc.sync.dma_start(out=w2c_t[:], in_=w2c_d[:])
            nc.sync.dma_start(out=ohm_t[:], in_=ohm_d[:])

            # ---- spbT = W1b^T @ supportT + b1   [2][128, SP] f32 ------
            # b1 is folded into the matmul as a K=1 rank-1 update
            # (lhsT = b1 row, rhs = ones row): TensorScalarPtr only has
            # one sync-wait slot in its HW struct, so a psum+bias add on
            # DVE is not encodable when the two inputs need two waits.
            spb_t = [cpool.tile([128, SP], f32, name=f"spb{i}") for i in range(2)]
            for db in range(2):
                sps = ppool.tile([128, QC], f32, tag="ps", name=f"sps{db}")
                nc.tensor.matmul(
                    sps[:, :SP],
                    w1b_t[0][:, 128 * db : 128 * (db + 1)],
                    sT_t[0][:],
                    start=True, stop=False,
                )
                nc.tensor.matmul(
                    sps[:, :SP],
                    w1b_t[1][:, 128 * db : 128 * (db + 1)],
                    sT_t[1][:],
                    start=False, stop=False,
                )
                nc.tensor.matmul(
                    sps[:, :SP],
                    b1_t[:, 128 * db : 128 * (db + 1)],
                    ones_t[:],
                    start=False, stop=True,
                )
                nc.vector.tensor_copy(out=spb_t[db][:], in_=sps[:, :SP])

            # ---- qpT = W1a^T @ queryT   [2][128, Q] bf16 --------------
            qpT_t = [spool.tile([128, Q], b16, name=f"qpT{i}") for i in range(2)]
            ncopy = 0
            for db in range(2):
                for qc in range(NQC):
                    qps = ppool.tile([128, QC], f32, tag="ps", name=f"qps{db}{qc}")
                    nc.tensor.matmul(
                        qps[:],
                        w1a_t[0][:, 128 * db : 128 * (db + 1)],
                        qT_t[0][:, QC * qc : QC * (qc + 1)],
                        start=True, stop=False,
                    )
                    nc.tensor.matmul(
                        qps[:],
                        w1a_t[1][:, 128 * db : 128 * (db + 1)],
                        qT_t[1][:, QC * qc : QC * (qc + 1)],
                        start=False, stop=True,
                    )
                    # All qpT copies stay on DVE so the main-loop DVE
                    # tensor_scalar ops see qpT as a same-engine producer
                    # (their single wait slot is needed for H recycling).
                    dst = qpT_t[db][:, QC * qc : QC * (qc + 1)]
                    nc.vector.tensor_copy(out=dst, in_=qps[:])
                    ncopy += 1

            # ---- main loop: H = relu(qpT + spb[:, s]);  scores += ----
            scores_ps = [
                ppool.tile([128, QC], f32, tag="ps", name=f"sc{qc}") for qc in range(NQC)
            ]
            ts_idx = 0
            for r in range(NR):
                h_tiles = {}
                for j in range(4):
                    sl = 4 * r + j
                    for db in range(2):
                        # Separate slot tags per producing engine: a tile
                        # whose slot previously held a same-engine tile
                        # needs only ONE cross-engine wait (PE readers) --
                        # the short AC/TS instruction structs have a
                        # single sync-wait slot.
                        use_act = ts_idx % 9 >= 7   # ~2/9 of ops on ACT
                        if use_act:
                            h = hpool.tile(
                                [128, Q], b16, tag="Ha", bufs=6, name=f"h{sl}_{db}"
                            )
                            nc.scalar.activation(
                                h[:], qpT_t[db][:], RELU,
                                bias=spb_t[db][:, sl : sl + 1],
                            )
                        else:
                            h = hpool.tile(
                                [128, Q], b16, tag="Hd", bufs=14, name=f"h{sl}_{db}"
                            )
                            nc.vector.tensor_scalar(
                                out=h[:], in0=qpT_t[db][:],
                                scalar1=spb_t[db][:, sl : sl + 1],
                                scalar2=0.0, op0=ADD, op1=MAX,
                            )
                        ts_idx += 1
                        h_tiles[(j, db)] = h
                for db in range(2):
                    w2blk = w2c_t[:, 32 * (db * NR + r) : 32 * (db * NR + r) + 32]
                    for qc in range(NQC):
                        for j in range(4):
                            nc.tensor.matmul(
                                scores_ps[qc][32 * j : 32 * j + 32, :],
                                w2blk,
                                h_tiles[(j, db)][:, QC * qc : QC * (qc + 1)],
                                start=(r == 0 and db == 0),
                                stop=(r == NR - 1 and db == 1),
                                tile_position=(0, 32 * j),
                                skip_group_check=True,
                            )

            # ---- E = exp(scores)  [128, Q] bf16 -----------------------
            e_t = spool.tile([128, Q], b16, name="et")
            for qc in range(NQC):
                nc.scalar.activation(
                    e_t[:, QC * qc : QC * (qc + 1)], scores_ps[qc][:], EXP,
                )

            # ---- part = [onehot | ones]^T @ E  -> [21, Q] f32 ---------
            out_sb = spool.tile([NWAY + 1, Q], f32, name="outsb")
            for qc in range(NQC):
                fps = ppool.tile([NWAY + 1, QC], f32, tag="ps", name=f"fps{qc}")
                nc.tensor.matmul(
                    fps[:], ohm_t[:], e_t[:, QC * qc : QC * (qc + 1)],
                    start=True, stop=True,
                )
                dst = out_sb[:, QC * qc : QC * (qc + 1)]
                if qc % 2 == 0:
                    nc.vector.tensor_copy(out=dst, in_=fps[:])
                else:
                    nc.scalar.copy(out=dst, in_=fps[:])
            nc.sync.dma_start(out=out_d[:], in_=out_sb[:])

    nc.finalize()
    return nc


def _host_prep(inputs):
    """Host-side layout prep: transposes, dtype casts, one-hot tables.

    Returns the list of 8 per-core input dicts for the bass kernel.
    """
    q = np.ascontiguousarray(np.asarray(inputs["query_embeddings"], dtype=np.float32))
    s = np.ascontiguousarray(np.asarray(inputs["support_embeddings"], dtype=np.float32))
    lab = np.asarray(inputs["support_labels"]).astype(np.int64)
    W1 = np.asarray(inputs["W1"], dtype=np.float32)
    b1 = np.asarray(inputs["b1"], dtype=np.float32)
    W2 = np.asarray(inputs["W2"], dtype=np.float32)

    qT = np.ascontiguousarray(q.T).astype(bf16)            # [D, Q]
    sT_full = np.ascontiguousarray(s.T).astype(bf16)       # [D, S]
    w1a = np.ascontiguousarray(W1[:D]).astype(bf16)        # [D, D] (din, dout)
    w1b = np.ascontiguousarray(W1[D:]).astype(bf16)
    b1r = np.ascontiguousarray(b1.reshape(1, D)).astype(bf16)  # [1, D]

    w2c = np.zeros((128, 2 * NR * 32), dtype=np.float32)
    for db in range(2):
        blk = W2[128 * db : 128 * (db + 1)]
        for r in range(NR):
            w2c[:, 32 * (db * NR + r) + r] = blk
    w2c = w2c.astype(bf16)

    in_maps = []
    for c in range(N_CORES):
        lo = c * SP
        ohm = np.zeros((128, NWAY + 1), dtype=np.float32)
        for sl in range(SP):
            row = 32 * (sl % 4) + sl // 4
            ohm[row, lab[lo + sl]] = 1.0
            ohm[row, NWAY] = 1.0
        in_maps.append(
            {
                "qT": qT,
                "w1a": w1a,
                "w1b": w1b,
                "sT": np.ascontiguousarray(sT_full[:, lo : lo + SP]),
                "b1r": b1r,
                "w2c": w2c,
                "ohm": ohm.astype(bf16),
            }
        )
    return in_maps


def _combine(parts):
    """Sum per-core partials and normalize -> [Q, NWAY] f32."""
    total = np.zeros((NWAY + 1, Q), dtype=np.float32)
    for p in parts:
        total += np.asarray(p, dtype=np.float32)
    return np.ascontiguousarray((total[:NWAY] / total[NWAY : NWAY + 1]).T)


def get_nc():
    global _compiled
    if _compiled is None:
        _compiled = _build_nc()
    return _compiled


def kernel(**inputs) -> np.ndarray:
    from concourse.bass_utils import run_bass_kernel_spmd

    nc = get_nc()
    in_maps = _host_prep(inputs)
    res = run_bass_kernel_spmd(nc, in_maps, list(range(N_CORES)))
    return _combine([res.results[c]["part"] for c in range(N_CORES)])
